# revision 18
# baseline (speedup 1.0000x reference)
"""Grouped-experts MLP (MoE) kernel for Trainium2, expert-parallel over 8 cores.

Problem: x[B=2, E=8, N=1024, D=1024]; per expert e:
    out[:, e] = GELU(x[:, e] @ w1[e] + b1[e]) @ w2[e] + b2[e]
with w1[e]: [D=1024, H=4096], w2[e]: [H=4096, D=1024].

Sharding: expert axis across the 8 NeuronCores (core e owns expert e).
The host performs the "all-to-all": it hands core e the slab x[:, e]
(pre-transposed to [D, T] so the contraction dim lands on SBUF partitions)
plus expert e's weights, and reassembles the full output afterward.

Per-core kernel (T = B*N = 2048 tokens):
  layer 1 computes hT[H, T] = w1.T @ xT in H-chunks of 512, GELU fused into
  the PSUM->SBUF eviction on the scalar engine (bias b1 is per-partition).
  layer 2 accumulates out[T, D] += hT_chunk.T-slices @ w2_chunk into an
  SBUF-resident accumulator via DVE adds; b2 (pre-broadcast on the host)
  is added by the first chunk's eviction.
All matmuls run as float32r (full fp32 data, 1 cycle/row at free-dim 512).
"""

import numpy as np

import concourse.bacc as bacc
import concourse.mybir as mybir
import concourse.tile as tile
from concourse.bass_utils import run_bass_kernel_spmd

B, E, N, D, H = 2, 8, 1024, 1024, 4096
T = B * N          # tokens per expert
P = 128
N_CORES = 8

T_HALF = 1024      # token half processed per outer iteration
H_CHUNK = 512      # H processed per inner chunk
N_TH = T // T_HALF           # 2
N_HC = H // H_CHUNK          # 8
KD = D // P                  # 8 k-tiles over D
HS = H_CHUNK // P            # 4 h-subtiles per chunk
TS = T_HALF // P             # 8 token subtiles per half
DC = D // 512                # 2 output column chunks

F32 = mybir.dt.float32
F32R = mybir.dt.float32r
GELU = mybir.ActivationFunctionType.Gelu


def build_nc():
    nc = bacc.Bacc("TRN2", target_bir_lowering=False, debug=False)

    xT = nc.dram_tensor("xT", [D, T], F32, kind="ExternalInput")
    w1 = nc.dram_tensor("w1", [D, H], F32, kind="ExternalInput")
    b1 = nc.dram_tensor("b1", [P, H // P], F32, kind="ExternalInput")
    w2 = nc.dram_tensor("w2", [H, D], F32, kind="ExternalInput")
    b2 = nc.dram_tensor("b2", [P, D], F32, kind="ExternalInput")
    out = nc.dram_tensor("out", [T, D], F32, kind="ExternalOutput")

    with tile.TileContext(nc) as tc:
        with (
            tc.tile_pool(name="const", bufs=1) as constp,
            tc.tile_pool(name="xTp", bufs=1) as xTp,
            tc.tile_pool(name="outp", bufs=1) as outp,
            tc.tile_pool(name="w1p", bufs=16) as w1p,
            tc.tile_pool(name="w2p", bufs=16) as w2p,
            tc.tile_pool(name="hTp", bufs=6) as hTp,
            tc.tile_pool(name="ps1p", bufs=4, space="PSUM") as ps1p,
            tc.tile_pool(name="ps2p", bufs=3, space="PSUM") as ps2p,
        ):
            def load_w1_chunk(h0):
                tiles = []
                for k in range(KD):
                    t = w1p.tile([P, H_CHUNK], F32R, name="w1t", tag="w1t")
                    nc.sync.dma_start(
                        t[:], w1[k * P:(k + 1) * P, h0:h0 + H_CHUNK].bitcast(F32R))
                    tiles.append(t)
                return tiles

            def load_xT_half(th):
                t0 = th * T_HALF
                tiles = []
                for k in range(KD):
                    t = xTp.tile([P, T_HALF], F32R,
                                 name=f"xT{k}_{th}", tag=f"xT{k}_{th}")
                    nc.gpsimd.dma_start(
                        t[:], xT[k * P:(k + 1) * P, t0:t0 + T_HALF].bitcast(F32R))
                    tiles.append(t)
                return tiles

            # startup: interleave first w1 chunk with the th0 activations so
            # the PE can start as soon as the first pair of tiles lands
            w1_pre = []
            xT0 = []
            for k in range(KD):
                t = w1p.tile([P, H_CHUNK], F32R, name="w1t", tag="w1t")
                nc.sync.dma_start(
                    t[:], w1[k * P:(k + 1) * P, 0:H_CHUNK].bitcast(F32R))
                w1_pre.append(t)
                t = xTp.tile([P, T_HALF], F32R, name=f"xT{k}_0", tag=f"xT{k}_0")
                nc.gpsimd.dma_start(
                    t[:], xT[k * P:(k + 1) * P, 0:T_HALF].bitcast(F32R))
                xT0.append(t)
            xT_half = [xT0, None]

            b1sb = constp.tile([P, H // P], F32, name="b1sb")
            nc.sync.dma_start(b1sb[:], b1[:])
            b2sb = constp.tile([P, D], F32, name="b2sb")
            nc.sync.dma_start(b2sb[:], b2[:])

            for th in range(N_TH):
                t0 = th * T_HALF
                xTs = xT_half[th]
                outs = []
                for ts in range(TS):
                    t = outp.tile([P, D], F32, name=f"out{ts}", tag=f"out{ts}")
                    outs.append(t)

                for hc in range(N_HC):
                    h0 = hc * H_CHUNK
                    if th == 0 and hc == 0:
                        w1t = w1_pre
                    else:
                        w1t = load_w1_chunk(h0)

                    # layer 1: hT chunk as HS tiles [128, T_HALF]; each
                    # 512-token group accumulates in one PSUM bank
                    hTt = []
                    for hs in range(HS):
                        ht = hTp.tile([P, T_HALF], F32R, name="hTt", tag="hTt")
                        for tq in range(T_HALF // 512):
                            p = ps1p.tile([P, 512], F32, name="ps1", tag="ps1")
                            for k in range(KD):
                                nc.tensor.matmul(
                                    p[:],
                                    w1t[k][:, hs * P:(hs + 1) * P],
                                    xTs[k][:, tq * 512:(tq + 1) * 512],
                                    start=(k == 0),
                                    stop=(k == KD - 1),
                                )
                            nc.scalar.activation(
                                ht[:, tq * 512:(tq + 1) * 512], p[:], GELU,
                                bias=b1sb[:, hc * HS + hs: hc * HS + hs + 1])
                        hTt.append(ht)

                    # stream w2 chunk
                    w2t = {}
                    for hs in range(HS):
                        for dc in range(DC):
                            t = w2p.tile([P, 512], F32R, name="w2t", tag="w2t")
                            nc.gpsimd.dma_start(
                                t[:],
                                w2[h0 + hs * P: h0 + (hs + 1) * P,
                                   dc * 512:(dc + 1) * 512].bitcast(F32R))
                            w2t[(hs, dc)] = t

                    # prefetch th1 activations once hc1's weights are queued
                    if th == 0 and hc == 1:
                        xT_half[1] = load_xT_half(1)

                    for ts in range(TS):
                        for dc in range(DC):
                            sl = slice(dc * 512, (dc + 1) * 512)
                            p = ps2p.tile([P, 512], F32, name="ps2", tag="ps2")
                            for hs in range(HS):
                                nc.tensor.matmul(
                                    p[:],
                                    hTt[hs][:, ts * P:(ts + 1) * P],
                                    w2t[(hs, dc)][:],
                                    start=(hs == 0),
                                    stop=(hs == HS - 1),
                                )
                            if hc == 0:
                                nc.vector.tensor_add(
                                    outs[ts][:, sl], b2sb[:, sl], p[:])
                            else:
                                nc.vector.tensor_add(
                                    outs[ts][:, sl], outs[ts][:, sl], p[:])
                            if hc == N_HC - 1:
                                nc.sync.dma_start(
                                    out[t0 + ts * P: t0 + (ts + 1) * P, sl],
                                    outs[ts][:, sl])

    nc.compile()
    return nc


def make_in_map(x_e, w1_e, b1_e, w2_e, b2_e):
    """Per-core input map from one expert's full-precision slabs."""
    xT = np.ascontiguousarray(x_e.reshape(T, D).T)
    return {
        "xT": xT,
        "w1": np.ascontiguousarray(w1_e),
        "b1": np.ascontiguousarray(b1_e.reshape(H // P, P).T),
        "w2": np.ascontiguousarray(w2_e),
        "b2": np.ascontiguousarray(
            np.broadcast_to(b2_e.reshape(1, D), (P, D))),
    }


_NC_CACHE = None


def _get_nc():
    global _NC_CACHE
    if _NC_CACHE is None:
        _NC_CACHE = build_nc()
    return _NC_CACHE


def kernel(x, w1, b1, w2, b2, trace=False):
    x = np.asarray(x, dtype=np.float32)
    w1 = np.asarray(w1, dtype=np.float32)
    b1 = np.asarray(b1, dtype=np.float32)
    w2 = np.asarray(w2, dtype=np.float32)
    b2 = np.asarray(b2, dtype=np.float32)

    nc = _get_nc()
    in_maps = [
        make_in_map(x[:, e], w1[e], b1[e], w2[e], b2[e]) for e in range(N_CORES)
    ]
    res = run_bass_kernel_spmd(
        nc, in_maps, core_ids=list(range(N_CORES)), trace=trace)
    out = np.empty((B, E, N, D), np.float32)
    for e in range(N_CORES):
        out[:, e] = res.results[e]["out"].reshape(B, N, D)
    if trace:
        return out, res
    return out


# revision 19
# speedup vs baseline: 1.0015x; 1.0015x over previous
"""Grouped-experts MLP (MoE) kernel for Trainium2, expert-parallel over 8 cores.

Problem: x[B=2, E=8, N=1024, D=1024]; per expert e:
    out[:, e] = GELU(x[:, e] @ w1[e] + b1[e]) @ w2[e] + b2[e]
with w1[e]: [D=1024, H=4096], w2[e]: [H=4096, D=1024].

Sharding: expert axis across the 8 NeuronCores (core e owns expert e).
The host performs the "all-to-all": it hands core e the slab x[:, e]
(pre-transposed to [D, T] so the contraction dim lands on SBUF partitions)
plus expert e's weights, and reassembles the full output afterward.

Per-core kernel (T = B*N = 2048 tokens):
  layer 1 computes hT[H, T] = w1.T @ xT in H-chunks of 512, GELU fused into
  the PSUM->SBUF eviction on the scalar engine (bias b1 is per-partition).
  layer 2 accumulates out[T, D] += hT_chunk.T-slices @ w2_chunk into an
  SBUF-resident accumulator via DVE adds; b2 (pre-broadcast on the host)
  is added by the first chunk's eviction.
All matmuls run as float32r (full fp32 data, 1 cycle/row at free-dim 512).
"""

import numpy as np

import concourse.bacc as bacc
import concourse.mybir as mybir
import concourse.tile as tile
from concourse.bass_utils import run_bass_kernel_spmd

B, E, N, D, H = 2, 8, 1024, 1024, 4096
T = B * N          # tokens per expert
P = 128
N_CORES = 8

T_HALF = 1024      # token half processed per outer iteration
H_CHUNK = 512      # H processed per inner chunk
N_TH = T // T_HALF           # 2
N_HC = H // H_CHUNK          # 8
KD = D // P                  # 8 k-tiles over D
HS = H_CHUNK // P            # 4 h-subtiles per chunk
TS = T_HALF // P             # 8 token subtiles per half
DC = D // 512                # 2 output column chunks

F32 = mybir.dt.float32
F32R = mybir.dt.float32r
GELU = mybir.ActivationFunctionType.Gelu


def build_nc():
    nc = bacc.Bacc("TRN2", target_bir_lowering=False, debug=False)

    xT = nc.dram_tensor("xT", [D, T], F32, kind="ExternalInput")
    w1 = nc.dram_tensor("w1", [D, H], F32, kind="ExternalInput")
    b1 = nc.dram_tensor("b1", [P, H // P], F32, kind="ExternalInput")
    w2 = nc.dram_tensor("w2", [H, D], F32, kind="ExternalInput")
    b2 = nc.dram_tensor("b2", [P, D], F32, kind="ExternalInput")
    out = nc.dram_tensor("out", [T, D], F32, kind="ExternalOutput")

    with tile.TileContext(nc) as tc:
        with (
            tc.tile_pool(name="const", bufs=1) as constp,
            tc.tile_pool(name="xTp", bufs=1) as xTp,
            tc.tile_pool(name="outp", bufs=1) as outp,
            tc.tile_pool(name="w1p", bufs=16) as w1p,
            tc.tile_pool(name="w2p", bufs=16) as w2p,
            tc.tile_pool(name="hTp", bufs=6) as hTp,
            tc.tile_pool(name="ps1p", bufs=4, space="PSUM") as ps1p,
            tc.tile_pool(name="ps2p", bufs=3, space="PSUM") as ps2p,
        ):
            def load_w1_chunk(h0):
                tiles = []
                for k in range(KD):
                    t = w1p.tile([P, H_CHUNK], F32R, name="w1t", tag="w1t")
                    nc.sync.dma_start(
                        t[:], w1[k * P:(k + 1) * P, h0:h0 + H_CHUNK].bitcast(F32R))
                    tiles.append(t)
                return tiles

            def load_xT_half(th):
                t0 = th * T_HALF
                tiles = []
                for k in range(KD):
                    t = xTp.tile([P, T_HALF], F32R,
                                 name=f"xT{k}_{th}", tag=f"xT{k}_{th}")
                    nc.sync.dma_start(
                        t[:], xT[k * P:(k + 1) * P, t0:t0 + T_HALF].bitcast(F32R))
                    tiles.append(t)
                return tiles

            # startup: interleave first w1 chunk with the th0 activations so
            # the PE can start as soon as the first pair of tiles lands
            w1_pre = []
            xT0 = []
            for k in range(KD):
                t = w1p.tile([P, H_CHUNK], F32R, name="w1t", tag="w1t")
                nc.sync.dma_start(
                    t[:], w1[k * P:(k + 1) * P, 0:H_CHUNK].bitcast(F32R))
                w1_pre.append(t)
                t = xTp.tile([P, T_HALF], F32R, name=f"xT{k}_0", tag=f"xT{k}_0")
                nc.sync.dma_start(
                    t[:], xT[k * P:(k + 1) * P, 0:T_HALF].bitcast(F32R))
                xT0.append(t)
            xT_half = [xT0, None]

            b1sb = constp.tile([P, H // P], F32, name="b1sb")
            nc.sync.dma_start(b1sb[:], b1[:])
            b2sb = constp.tile([P, D], F32, name="b2sb")
            nc.sync.dma_start(b2sb[:], b2[:])

            for th in range(N_TH):
                t0 = th * T_HALF
                xTs = xT_half[th]
                outs = []
                for ts in range(TS):
                    t = outp.tile([P, D], F32, name=f"out{ts}", tag=f"out{ts}")
                    outs.append(t)

                for hc in range(N_HC):
                    h0 = hc * H_CHUNK
                    if th == 0 and hc == 0:
                        w1t = w1_pre
                    else:
                        w1t = load_w1_chunk(h0)

                    # layer 1: hT chunk as HS tiles [128, T_HALF]; each
                    # 512-token group accumulates in one PSUM bank
                    hTt = []
                    for hs in range(HS):
                        ht = hTp.tile([P, T_HALF], F32R, name="hTt", tag="hTt")
                        for tq in range(T_HALF // 512):
                            p = ps1p.tile([P, 512], F32, name="ps1", tag="ps1")
                            for k in range(KD):
                                nc.tensor.matmul(
                                    p[:],
                                    w1t[k][:, hs * P:(hs + 1) * P],
                                    xTs[k][:, tq * 512:(tq + 1) * 512],
                                    start=(k == 0),
                                    stop=(k == KD - 1),
                                )
                            nc.scalar.activation(
                                ht[:, tq * 512:(tq + 1) * 512], p[:], GELU,
                                bias=b1sb[:, hc * HS + hs: hc * HS + hs + 1])
                        hTt.append(ht)

                    # stream w2 chunk
                    w2t = {}
                    for hs in range(HS):
                        for dc in range(DC):
                            t = w2p.tile([P, 512], F32R, name="w2t", tag="w2t")
                            nc.sync.dma_start(
                                t[:],
                                w2[h0 + hs * P: h0 + (hs + 1) * P,
                                   dc * 512:(dc + 1) * 512].bitcast(F32R))
                            w2t[(hs, dc)] = t

                    # prefetch th1 activations once hc1's weights are queued
                    if th == 0 and hc == 1:
                        xT_half[1] = load_xT_half(1)

                    for ts in range(TS):
                        for dc in range(DC):
                            sl = slice(dc * 512, (dc + 1) * 512)
                            p = ps2p.tile([P, 512], F32, name="ps2", tag="ps2")
                            for hs in range(HS):
                                nc.tensor.matmul(
                                    p[:],
                                    hTt[hs][:, ts * P:(ts + 1) * P],
                                    w2t[(hs, dc)][:],
                                    start=(hs == 0),
                                    stop=(hs == HS - 1),
                                )
                            if hc == 0:
                                nc.vector.tensor_add(
                                    outs[ts][:, sl], b2sb[:, sl], p[:])
                            else:
                                nc.vector.tensor_add(
                                    outs[ts][:, sl], outs[ts][:, sl], p[:])
                            if hc == N_HC - 1:
                                nc.sync.dma_start(
                                    out[t0 + ts * P: t0 + (ts + 1) * P, sl],
                                    outs[ts][:, sl])

    nc.compile()
    return nc


def make_in_map(x_e, w1_e, b1_e, w2_e, b2_e):
    """Per-core input map from one expert's full-precision slabs."""
    xT = np.ascontiguousarray(x_e.reshape(T, D).T)
    return {
        "xT": xT,
        "w1": np.ascontiguousarray(w1_e),
        "b1": np.ascontiguousarray(b1_e.reshape(H // P, P).T),
        "w2": np.ascontiguousarray(w2_e),
        "b2": np.ascontiguousarray(
            np.broadcast_to(b2_e.reshape(1, D), (P, D))),
    }


_NC_CACHE = None


def _get_nc():
    global _NC_CACHE
    if _NC_CACHE is None:
        _NC_CACHE = build_nc()
    return _NC_CACHE


def kernel(x, w1, b1, w2, b2, trace=False):
    x = np.asarray(x, dtype=np.float32)
    w1 = np.asarray(w1, dtype=np.float32)
    b1 = np.asarray(b1, dtype=np.float32)
    w2 = np.asarray(w2, dtype=np.float32)
    b2 = np.asarray(b2, dtype=np.float32)

    nc = _get_nc()
    in_maps = [
        make_in_map(x[:, e], w1[e], b1[e], w2[e], b2[e]) for e in range(N_CORES)
    ]
    res = run_bass_kernel_spmd(
        nc, in_maps, core_ids=list(range(N_CORES)), trace=trace)
    out = np.empty((B, E, N, D), np.float32)
    for e in range(N_CORES):
        out[:, e] = res.results[e]["out"].reshape(B, N, D)
    if trace:
        return out, res
    return out


# revision 20
# speedup vs baseline: 1.0044x; 1.0029x over previous
"""Grouped-experts MLP (MoE) kernel for Trainium2, expert-parallel over 8 cores.

Problem: x[B=2, E=8, N=1024, D=1024]; per expert e:
    out[:, e] = GELU(x[:, e] @ w1[e] + b1[e]) @ w2[e] + b2[e]
with w1[e]: [D=1024, H=4096], w2[e]: [H=4096, D=1024].

Sharding: expert axis across the 8 NeuronCores (core e owns expert e).
The host performs the "all-to-all": it hands core e the slab x[:, e]
(pre-transposed to [D, T] so the contraction dim lands on SBUF partitions)
plus expert e's weights, and reassembles the full output afterward.

Per-core kernel (T = B*N = 2048 tokens):
  layer 1 computes hT[H, T] = w1.T @ xT in H-chunks of 512, GELU fused into
  the PSUM->SBUF eviction on the scalar engine (bias b1 is per-partition).
  layer 2 accumulates out[T, D] += hT_chunk.T-slices @ w2_chunk into an
  SBUF-resident accumulator via DVE adds; b2 (pre-broadcast on the host)
  is added by the first chunk's eviction.
All matmuls run as float32r (full fp32 data, 1 cycle/row at free-dim 512).
"""

import numpy as np

import concourse.bacc as bacc
import concourse.mybir as mybir
import concourse.tile as tile
from concourse.bass_utils import run_bass_kernel_spmd

B, E, N, D, H = 2, 8, 1024, 1024, 4096
T = B * N          # tokens per expert
P = 128
N_CORES = 8

T_HALF = 1024      # token half processed per outer iteration
H_CHUNK = 512      # H processed per inner chunk
N_TH = T // T_HALF           # 2
N_HC = H // H_CHUNK          # 8
KD = D // P                  # 8 k-tiles over D
HS = H_CHUNK // P            # 4 h-subtiles per chunk
TS = T_HALF // P             # 8 token subtiles per half
DC = D // 512                # 2 output column chunks

F32 = mybir.dt.float32
F32R = mybir.dt.float32r
GELU = mybir.ActivationFunctionType.Gelu


def build_nc():
    nc = bacc.Bacc("TRN2", target_bir_lowering=False, debug=False)

    xT = nc.dram_tensor("xT", [D, T], F32, kind="ExternalInput")
    w1 = nc.dram_tensor("w1", [D, H], F32, kind="ExternalInput")
    b1 = nc.dram_tensor("b1", [P, H // P], F32, kind="ExternalInput")
    w2 = nc.dram_tensor("w2", [H, D], F32, kind="ExternalInput")
    b2 = nc.dram_tensor("b2", [P, D], F32, kind="ExternalInput")
    out = nc.dram_tensor("out", [T, D], F32, kind="ExternalOutput")

    with tile.TileContext(nc) as tc:
        with (
            tc.tile_pool(name="const", bufs=1) as constp,
            tc.tile_pool(name="xTp", bufs=1) as xTp,
            tc.tile_pool(name="outp", bufs=1) as outp,
            tc.tile_pool(name="w1p", bufs=12) as w1p,
            tc.tile_pool(name="w2p", bufs=16) as w2p,
            tc.tile_pool(name="hTp", bufs=8) as hTp,
            tc.tile_pool(name="ps1p", bufs=4, space="PSUM") as ps1p,
            tc.tile_pool(name="ps2p", bufs=3, space="PSUM") as ps2p,
        ):
            def load_w1_chunk(h0):
                tiles = []
                for k in range(KD):
                    t = w1p.tile([P, H_CHUNK], F32R, name="w1t", tag="w1t")
                    nc.sync.dma_start(
                        t[:], w1[k * P:(k + 1) * P, h0:h0 + H_CHUNK].bitcast(F32R))
                    tiles.append(t)
                return tiles

            def load_xT_half(th):
                t0 = th * T_HALF
                tiles = []
                for k in range(KD):
                    t = xTp.tile([P, T_HALF], F32R,
                                 name=f"xT{k}_{th}", tag=f"xT{k}_{th}")
                    nc.sync.dma_start(
                        t[:], xT[k * P:(k + 1) * P, t0:t0 + T_HALF].bitcast(F32R))
                    tiles.append(t)
                return tiles

            # startup: interleave first w1 chunk with the th0 activations so
            # the PE can start as soon as the first pair of tiles lands
            w1_pre = []
            xT0 = []
            for k in range(KD):
                t = w1p.tile([P, H_CHUNK], F32R, name="w1t", tag="w1t")
                nc.sync.dma_start(
                    t[:], w1[k * P:(k + 1) * P, 0:H_CHUNK].bitcast(F32R))
                w1_pre.append(t)
                t = xTp.tile([P, T_HALF], F32R, name=f"xT{k}_0", tag=f"xT{k}_0")
                nc.sync.dma_start(
                    t[:], xT[k * P:(k + 1) * P, 0:T_HALF].bitcast(F32R))
                xT0.append(t)
            xT_half = [xT0, None]

            b1sb = constp.tile([P, H // P], F32, name="b1sb")
            nc.sync.dma_start(b1sb[:], b1[:])
            b2sb = constp.tile([P, D], F32, name="b2sb")
            nc.sync.dma_start(b2sb[:], b2[:])

            for th in range(N_TH):
                t0 = th * T_HALF
                xTs = xT_half[th]
                outs = []
                for ts in range(TS):
                    t = outp.tile([P, D], F32, name=f"out{ts}", tag=f"out{ts}")
                    outs.append(t)

                for hc in range(N_HC):
                    h0 = hc * H_CHUNK
                    if th == 0 and hc == 0:
                        w1t = w1_pre
                    else:
                        w1t = load_w1_chunk(h0)

                    # layer 1: hT chunk as HS tiles [128, T_HALF]; each
                    # 512-token group accumulates in one PSUM bank
                    hTt = []
                    for hs in range(HS):
                        ht = hTp.tile([P, T_HALF], F32R, name="hTt", tag="hTt")
                        for tq in range(T_HALF // 512):
                            p = ps1p.tile([P, 512], F32, name="ps1", tag="ps1")
                            for k in range(KD):
                                nc.tensor.matmul(
                                    p[:],
                                    w1t[k][:, hs * P:(hs + 1) * P],
                                    xTs[k][:, tq * 512:(tq + 1) * 512],
                                    start=(k == 0),
                                    stop=(k == KD - 1),
                                )
                            nc.scalar.activation(
                                ht[:, tq * 512:(tq + 1) * 512], p[:], GELU,
                                bias=b1sb[:, hc * HS + hs: hc * HS + hs + 1])
                        hTt.append(ht)

                    # stream w2 chunk
                    w2t = {}
                    for hs in range(HS):
                        for dc in range(DC):
                            t = w2p.tile([P, 512], F32R, name="w2t", tag="w2t")
                            nc.sync.dma_start(
                                t[:],
                                w2[h0 + hs * P: h0 + (hs + 1) * P,
                                   dc * 512:(dc + 1) * 512].bitcast(F32R))
                            w2t[(hs, dc)] = t

                    # prefetch th1 activations once hc1's weights are queued
                    if th == 0 and hc == 1:
                        xT_half[1] = load_xT_half(1)

                    for ts in range(TS):
                        for dc in range(DC):
                            sl = slice(dc * 512, (dc + 1) * 512)
                            p = ps2p.tile([P, 512], F32, name="ps2", tag="ps2")
                            for hs in range(HS):
                                nc.tensor.matmul(
                                    p[:],
                                    hTt[hs][:, ts * P:(ts + 1) * P],
                                    w2t[(hs, dc)][:],
                                    start=(hs == 0),
                                    stop=(hs == HS - 1),
                                )
                            if hc == 0:
                                nc.vector.tensor_add(
                                    outs[ts][:, sl], b2sb[:, sl], p[:])
                            else:
                                nc.vector.tensor_add(
                                    outs[ts][:, sl], outs[ts][:, sl], p[:])
                            if hc == N_HC - 1:
                                nc.sync.dma_start(
                                    out[t0 + ts * P: t0 + (ts + 1) * P, sl],
                                    outs[ts][:, sl])

    nc.compile()
    return nc


def make_in_map(x_e, w1_e, b1_e, w2_e, b2_e):
    """Per-core input map from one expert's full-precision slabs."""
    xT = np.ascontiguousarray(x_e.reshape(T, D).T)
    return {
        "xT": xT,
        "w1": np.ascontiguousarray(w1_e),
        "b1": np.ascontiguousarray(b1_e.reshape(H // P, P).T),
        "w2": np.ascontiguousarray(w2_e),
        "b2": np.ascontiguousarray(
            np.broadcast_to(b2_e.reshape(1, D), (P, D))),
    }


_NC_CACHE = None


def _get_nc():
    global _NC_CACHE
    if _NC_CACHE is None:
        _NC_CACHE = build_nc()
    return _NC_CACHE


def kernel(x, w1, b1, w2, b2, trace=False):
    x = np.asarray(x, dtype=np.float32)
    w1 = np.asarray(w1, dtype=np.float32)
    b1 = np.asarray(b1, dtype=np.float32)
    w2 = np.asarray(w2, dtype=np.float32)
    b2 = np.asarray(b2, dtype=np.float32)

    nc = _get_nc()
    in_maps = [
        make_in_map(x[:, e], w1[e], b1[e], w2[e], b2[e]) for e in range(N_CORES)
    ]
    res = run_bass_kernel_spmd(
        nc, in_maps, core_ids=list(range(N_CORES)), trace=trace)
    out = np.empty((B, E, N, D), np.float32)
    for e in range(N_CORES):
        out[:, e] = res.results[e]["out"].reshape(B, N, D)
    if trace:
        return out, res
    return out
